# revision 18
# baseline (speedup 1.0000x reference)
"""Masked dot-product attention on 8 Trainium2 NeuronCores.

Problem: B=8, S=4096, D=64 fp32; per-batch key-length mask; softmax over keys.

Sharding: sequence-parallel over Q rows. Each core computes a 512-row Q slice
of all 8 batches. The key loop for batch b runs ceil(valid_len[b]/128) tiles
(same trip counts on every core -> one SPMD program, perfectly balanced
regardless of the valid_len distribution).

The kernel is one FLAT pipeline over all (batch, k-tile) pairs, chunked into
groups of 3; groups freely span batch boundaries so the softmax-exp engines
never stall at batch transitions. Per k-tile, scores kept in [k, q] layout:
  phase 1: psum_s[k=128, q=512] = K_tile.T @ Q in 64x128 PE-tiling mode --
           contraction is only D=64, so tile t goes to array rows (t%2)*64
           and CONSECUTIVE TILES RUN CONCURRENTLY in the two row-halves
           (Q is duplicated on partitions 64..127 to feed the upper half).
  exp:     4 of every 5 groups: one ScalarE activation, exp(s), PSUM -> SBUF
           bf16 (scores pre-scaled by 1/8 on the host, folded into Q: 2^-3;
           no max-subtraction needed, scores ~ N(0,1)).
           every 5th group: VectorE computes a two-phase Schraudolph exp --
           i16 = round(s*128*log2e + BIAS) is the bf16 bit pattern of
           ~2^(y-1); adding 64 in the int domain gives the half-period
           phase shift; summing the two bf16 views averages the scallop
           down to ~0.9% and BIAS is gain-calibrated so the approximation
           is unbiased vs the exact-exp tiles (softmax mixes them).
  phase 2: psum_o[72, q=512] += V_tile.T @ exp_tile in bf16. V is padded to
           72 weight columns, col 64 = ones, so row 64 of psum_o accumulates
           the softmax denominator. Adjacent batches alternate between two
           psum_o accumulator banks.
  tail:    DVE-copy psum_o[0:65] -> SBUF, DMA out. The divide (numerator /
           denominator) happens on the HOST.

Masking costs nothing on-device: the host zeroes V rows (incl. the ones
column) at key positions >= valid_len, so masked keys contribute 0 to both
numerator and denominator; exp of their scores is finite garbage times zero.

Perf notes: this box's PE activity governor throttles fully-dense matmul
streams to 1.2 GHz but tolerates ~90-95% duty at 2.4 GHz, so PE *cycles*
are minimized (phase-1 row-packing) while keeping small scheduling gaps.
K/Q/V ride the Sync HWDGE queue with flat 2D access patterns; k/v/q SBUF
buffers are rings (bufs=2/3) so DMA triggers self-pace one batch ahead;
largest batches first so the exposed tail batch is small.
"""

import math
from contextlib import ExitStack

import numpy as np

B = 8
S = 4096
D = 64
N_CORES = 8
QB = S // N_CORES  # 512 q rows per core per batch
KT = 128  # k rows per tile
NKMAX = S // KT  # 32
NPMAX = NKMAX // 2  # 16 k-tile pairs
VC = 72  # V weight columns: 64 value dims + 1 ones col + 7 pad
GROUP = 2  # k-tiles per PSUM group / exp instruction
DVE_EVERY = 3  # every 3rd group's exp runs on VectorE instead of ScalarE
SCALE = 1.0 / math.sqrt(D)  # 1/8, exact in bf16
EXP_A = 1.4426950408889634 * 128.0  # log2(e) * 2^7
EXP_B = 16256.0 - 128.0 - 32.0 - 10.118  # bf16 bias, /2 fold, phase center,
#                                          gain calibration (unbiased vs exp)

_PROGRAM_CACHE: dict = {}


def _build_program(k_tiles):
    import concourse.tile as tile
    from concourse import bacc, mybir

    f32 = mybir.dt.float32
    bf16 = mybir.dt.bfloat16
    i16 = mybir.dt.int16
    nc = bacc.Bacc("TRN2", target_bir_lowering=False, debug=False,
                   enable_asserts=False, num_devices=N_CORES)

    qx = nc.dram_tensor("qx", [B, KT, QB], bf16, kind="ExternalInput").ap()
    kx = nc.dram_tensor("kx", [B, KT, NPMAX * KT], bf16,
                        kind="ExternalInput").ap()
    vx = nc.dram_tensor("vx", [B, KT, NKMAX * VC], bf16,
                        kind="ExternalInput").ap()
    out = nc.dram_tensor("out", [B, D + 1, QB], f32, kind="ExternalOutput").ap()

    order = sorted(range(B), key=lambda x: -k_tiles[x])
    flat = [(b, t) for b in order for t in range(k_tiles[b])]
    ngroups = (len(flat) + GROUP - 1) // GROUP

    with tile.TileContext(nc) as tc:
        with ExitStack() as ctx:
            q_pool = ctx.enter_context(tc.tile_pool(name="q", bufs=3))
            k_pool = ctx.enter_context(tc.tile_pool(name="k", bufs=2))
            v_pool = ctx.enter_context(tc.tile_pool(name="v", bufs=2))
            e_pool = ctx.enter_context(tc.tile_pool(name="e", bufs=3))
            s_pool = ctx.enter_context(tc.tile_pool(name="s", bufs=2))
            o_pool = ctx.enter_context(tc.tile_pool(name="o", bufs=2))
            ps_s_pool = ctx.enter_context(
                tc.tile_pool(name="ps_s", bufs=3, space="PSUM"))
            ps_o_pool = ctx.enter_context(
                tc.tile_pool(name="ps_o", bufs=2, space="PSUM"))

            kt_sb = {}
            vt_sb = {}
            qt_sb = {}
            pso = {}
            e_tiles = {}

            def load_qk(b, split_first=False):
                np_b = (k_tiles[b] + 1) // 2
                qt = q_pool.tile([KT, QB], bf16, name=f"q{b}", tag="q")
                nc.sync.dma_start(qt[:], qx[b])
                k_all = k_pool.tile([KT, NPMAX * KT], bf16, name=f"k{b}",
                                    tag="k")
                if split_first and np_b > 1:
                    # first pair lands immediately so phase 1 starts early
                    nc.sync.dma_start(k_all[:, :KT], kx[b][:, :KT])
                    nc.sync.dma_start(k_all[:, KT:np_b * KT],
                                      kx[b][:, KT:np_b * KT])
                else:
                    nc.sync.dma_start(k_all[:, :np_b * KT],
                                      kx[b][:, :np_b * KT])
                qt_sb[b], kt_sb[b] = qt, k_all

            def load_v(b, split_first=False):
                nk = k_tiles[b]
                v_all = v_pool.tile([KT, NKMAX * VC], bf16, name=f"v{b}",
                                    tag="v")
                if split_first and nk > 8:
                    # first 8 tiles' completion semaphore fires early so the
                    # first phase-2 matmuls don't wait on the whole transfer
                    nc.sync.dma_start(v_all[:, :8 * VC], vx[b][:, :8 * VC])
                    nc.sync.dma_start(v_all[:, 8 * VC:nk * VC],
                                      vx[b][:, 8 * VC:nk * VC])
                else:
                    nc.sync.dma_start(v_all[:, :nk * VC], vx[b][:, :nk * VC])
                vt_sb[b] = v_all

            # q+k of the first two batches land before any V bytes so the
            # first phase-1 matmuls start as early as possible.
            load_qk(order[0], split_first=True)
            load_v(order[0], split_first=True)
            load_qk(order[1])
            load_v(order[1])
            next_load = 2

            def emit_p2s(g):
                gt = flat[g * GROUP:(g + 1) * GROUP]
                e_sb = e_tiles.pop(g)
                for i, (b, t) in enumerate(gt):
                    if t == 0:
                        pso[b] = ps_o_pool.tile([KT, QB], f32,
                                                name=f"pso{b}", tag="ps_o")
                    nc.tensor.matmul(
                        pso[b][:VC, :],
                        lhsT=vt_sb[b][:, t * VC:(t + 1) * VC],
                        rhs=e_sb[:, i * QB:(i + 1) * QB],
                        start=(t == 0), stop=(t == k_tiles[b] - 1),
                        skip_group_check=True)
                    if t == k_tiles[b] - 1:
                        o_n = o_pool.tile([D + 1, QB], f32, name=f"o{b}",
                                          tag="o_n", bufs=2)
                        nc.scalar.copy(o_n[:], pso[b][:D + 1, :])
                        nc.sync.dma_start(out[b], o_n[:])

            LAG = 4  # P2(g) is emitted at iteration g+LAG: the exp engines
            #          get ~LAG group-periods of latency slack, and the PE
            #          queue never stalls on an in-flight exp.
            def emit_p1s(g):
                gt = flat[g * GROUP:(g + 1) * GROUP]
                # prefetch the next batch when a new batch first appears
                for (b, t) in gt:
                    if t == 0 and b != order[0] and next_load[0] < B:
                        load_qk(order[next_load[0]])
                        load_v(order[next_load[0]])
                        next_load[0] += 1
                ps_s = ps_s_pool.tile([KT, GROUP * QB], f32, name="ps_s")
                for i, (b, t) in enumerate(gt):
                    p, half = divmod(t, 2)
                    lo = 64 * half
                    nc.tensor.matmul(
                        ps_s[:, i * QB:(i + 1) * QB],
                        lhsT=kt_sb[b][lo:lo + 64, p * KT:(p + 1) * KT],
                        rhs=qt_sb[b][lo:lo + 64, :],
                        start=True, stop=True)
                return ps_s

            def emit_exp(g, ps_s):
                gt = flat[g * GROUP:(g + 1) * GROUP]
                n = len(gt) * QB
                e_sb = e_pool.tile([KT, GROUP * QB], bf16, name="e_sb")
                e_tiles[g] = e_sb
                if g % DVE_EVERY == DVE_EVERY - 1 and g < ngroups - 6:
                    # two-phase Schraudolph exp on VectorE
                    s1 = s_pool.tile([KT, GROUP * QB], i16, name="s1",
                                     tag="s1")
                    s2 = s_pool.tile([KT, GROUP * QB], i16, name="s2",
                                     tag="s2")
                    nc.vector.tensor_scalar(
                        s1[:, :n], ps_s[:, :n], EXP_A, EXP_B,
                        mybir.AluOpType.mult, mybir.AluOpType.add)
                    nc.vector.tensor_scalar_add(s2[:, :n], s1[:, :n], 64)
                    nc.vector.tensor_add(
                        e_sb[:, :n], s1[:, :n].bitcast(bf16),
                        s2[:, :n].bitcast(bf16))
                else:
                    nc.scalar.activation(
                        e_sb[:, :n], ps_s[:, :n],
                        mybir.ActivationFunctionType.Exp)

            # two groups per iteration: P1s of both, then both exps, then
            # both lagged P2 blocks -- halves the PE's 64x128 <-> 128x128
            # weight-mode switches.
            next_load = [next_load]
            for g0 in range(0, ngroups, 2):
                gs = [g for g in (g0, g0 + 1) if g < ngroups]
                pss = [emit_p1s(g) for g in gs]
                for g, ps_s in zip(gs, pss):
                    emit_exp(g, ps_s)
                for g in gs:
                    if g >= LAG:
                        emit_p2s(g - LAG)
            for g in range(max(0, ngroups - LAG), ngroups):
                emit_p2s(g)

    nc.compile()
    return nc


def _prep_inputs(query, key, value, valid):
    import ml_dtypes

    vclamp = np.clip(valid, 1, S)
    k_tiles = tuple(int(x) for x in np.ceil(vclamp / KT).astype(np.int64))

    # K packed for 64x128 row-tiling: pair p holds k-tile 2p on partitions
    # 0..63 and k-tile 2p+1 on partitions 64..127, at columns [128p, 128p+128).
    kt4 = key.reshape(B, NPMAX, 2, KT, D)  # [B, pair, half, key, d]
    kxh = np.ascontiguousarray(
        kt4.transpose(0, 2, 4, 1, 3).reshape(B, KT, NPMAX * KT)
    ).astype(ml_dtypes.bfloat16)

    vxh = np.zeros((B, S, VC), dtype=np.float32)  # padded to 72 weight cols
    vxh[:, :, :D] = value
    vxh[:, :, D] = 1.0
    for b in range(B):
        vxh[b, vclamp[b]:, :] = 0.0  # masked keys contribute nothing
    # [B, S, 72] -> [B, KT, NKMAX*72]: per-partition contiguous k-tile runs
    vxt = np.ascontiguousarray(
        vxh.reshape(B, NKMAX, KT, VC).transpose(0, 2, 1, 3).reshape(
            B, KT, NKMAX * VC)
    ).astype(ml_dtypes.bfloat16)

    # Q scaled by 1/sqrt(D) (exact power of two) and duplicated onto
    # partitions 64..127 to feed the upper row-half of the PE array.
    qs = (query * SCALE).transpose(0, 2, 1)  # [B, D, S]

    in_maps = []
    for c in range(N_CORES):
        qc = qs[:, :, c * QB:(c + 1) * QB]  # [B, D, QB]
        qxh = np.concatenate([qc, qc], axis=1).astype(ml_dtypes.bfloat16)
        in_maps.append({"qx": np.ascontiguousarray(qxh),
                        "kx": kxh, "vx": vxt})
    return k_tiles, in_maps


def kernel(query, key, value, valid_len):
    from concourse.bass_utils import run_bass_kernel_spmd

    query = np.ascontiguousarray(query, dtype=np.float32)
    key = np.ascontiguousarray(key, dtype=np.float32)
    value = np.ascontiguousarray(value, dtype=np.float32)
    valid = np.asarray(valid_len).astype(np.int64)
    assert query.shape == (B, S, D) and key.shape == (B, S, D)
    assert value.shape == (B, S, D) and valid.shape == (B,)

    k_tiles, in_maps = _prep_inputs(query, key, value, valid)

    nc = _PROGRAM_CACHE.get(k_tiles)
    if nc is None:
        nc = _build_program(k_tiles)
        _PROGRAM_CACHE[k_tiles] = nc

    res = run_bass_kernel_spmd(nc, in_maps, core_ids=list(range(N_CORES)))

    full = np.empty((B, S, D), dtype=np.float32)
    for c in range(N_CORES):
        o = res.results[c]["out"]  # [B, 65, QB]: numerator rows + denom row
        full[:, c * QB:(c + 1) * QB, :] = (
            o[:, :D, :] / o[:, D:D + 1, :]).transpose(0, 2, 1)

    # valid_len == 0 never occurs per the spec (randint >= 1), but the
    # reference would produce uniform attention there; match it exactly.
    if np.any(valid < 1):
        for b in np.nonzero(valid < 1)[0]:
            sc = (query[b] @ key[b].T) * SCALE - 1.0e6
            a = np.exp(sc - sc.max(axis=-1, keepdims=True))
            a /= a.sum(axis=-1, keepdims=True)
            full[b] = a @ value[b]

    return full
